# revision 11
# baseline (speedup 1.0000x reference)
"""Multi-head attention (B=4, T=2048, D=2048, H=16, E=128) on 8 TRN2 NeuronCores.

Sharding: batch (4) x head-group (2 groups of 8 heads) -> 8 cores.
Per core: q/k/v projections for its 8 heads + softmax(QK^T/sqrt(E))V.

Layout strategy (no on-chip transposes are ever needed):
  - host passes x^T [D,T] and per-head W^T [D,E] (bf16) so the contraction
    dim D lands on SBUF partitions directly.
  - Q^T,K^T computed as [E,T] (lhsT=W^T chunk, rhs=x^T chunk).
  - V computed as [T,E] (lhsT=x^T chunk, rhs=Wv^T chunk), head-quads at N=512.
  - scores computed transposed: S^T[k,q] = (K Q^T), softmax-exp on ACT,
    P^T feeds C^T[e,q] = V^T P^T; denominators: DVE tree-sums the 16 P^T
    tiles of a q-chunk (bf16), then gpsimd.partition_all_reduce sums over
    partitions (zero PE cycles).
  - output written as C^T [h,E,T]; host transposes back to [h,T,E].

Scheduling: the attention inner loop alone is ACT-bound (exp of [128,512]
~446ns vs ~360ns of PE st/ct work per (qc,kt) slot). Measured on hw, the
4-instruction slot [st, ct, proj, proj] runs at the same ~178ns/instr as
the plain attn pair, so each attention slot of head h absorbs exactly two
projection matmuls of head h+1 from a sequential op stream - the Q/K
projections of heads 1..7 ride inside attention for free and the ACT exp
stream is fully hidden behind PE work for heads 0..6.
"""

import itertools
import math
import sys

sys.path.insert(0, "/opt/trn_rl_repo")

import ml_dtypes
import numpy as np

import concourse.bass as bass  # noqa: F401  (registers engine methods)
import concourse.mybir as mybir
import concourse.tile as tile
from concourse import bacc
from concourse import bass_isa
from concourse.bass_utils import run_bass_kernel_spmd

B, T, D, H, E = 4, 2048, 2048, 16, 128
N_CORES = 8
H_LOC = H // 2          # heads per core
P = 128                 # partitions
DT = D // P             # contraction chunks for projections
KT = T // P             # key tiles
QW = 512                # q-chunk width (one PSUM bank of fp32)
QC = T // QW
BF16 = mybir.dt.bfloat16
F32 = mybir.dt.float32
EXP_SCALE = 1.0 / math.sqrt(E)


def _build(repeat=1):
    nc = bacc.Bacc("TRN2", target_bir_lowering=False, debug=False,
                   num_devices=N_CORES)
    xT = nc.dram_tensor("xT", [D, T], BF16, kind="ExternalInput").ap()
    wqT = nc.dram_tensor("wqT", [H_LOC, D, E], BF16, kind="ExternalInput").ap()
    wkT = nc.dram_tensor("wkT", [H_LOC, D, E], BF16, kind="ExternalInput").ap()
    wvT = nc.dram_tensor("wvT", [H_LOC // 4, D, 4 * E], BF16,
                         kind="ExternalInput").ap()
    out = nc.dram_tensor("out", [H_LOC, E, T], F32, kind="ExternalOutput").ap()

    with tile.TileContext(nc) as tc:
        with (
            tc.tile_pool(name="xpool", bufs=1) as xpool,
            tc.tile_pool(name="wqk", bufs=2) as wqk,
            tc.tile_pool(name="wvp", bufs=2) as wvp,
            tc.tile_pool(name="qk", bufs=2) as qk,
            tc.tile_pool(name="vpool", bufs=2) as vpool,
            tc.tile_pool(name="ptp", bufs=8) as ptp,
            tc.tile_pool(name="outp", bufs=3) as outp,
            tc.tile_pool(name="smallp", bufs=3) as smallp,
            tc.tile_pool(name="dsum", bufs=2) as dsum,
            tc.tile_pool(name="onesp", bufs=1) as onesp,
            tc.tile_pool(name="stps", bufs=3, space="PSUM") as stps,
            tc.tile_pool(name="projps", bufs=2, space="PSUM") as projps,
            tc.tile_pool(name="ctps", bufs=2, space="PSUM") as ctps,
            tc.tile_pool(name="sumps", bufs=1, space="PSUM") as sumps,
        ):
            pools = dict(xpool=xpool, wqk=wqk, wvp=wvp, qk=qk, vpool=vpool,
                         ptp=ptp, outp=outp, smallp=smallp, dsum=dsum,
                         onesp=onesp, stps=stps, projps=projps, ctps=ctps,
                         sumps=sumps)
            for _rep in range(repeat):
                _kernel_rep(tc, nc, pools, xT, wqT, wkT, wvT, out)
    nc.compile()
    return nc


def _kernel_rep(tc, nc, pools, xT, wqT, wkT, wvT, out):
    xpool = pools["xpool"]; wqk = pools["wqk"]; wvp = pools["wvp"]
    qk = pools["qk"]; vpool = pools["vpool"]; ptp = pools["ptp"]
    outp = pools["outp"]; smallp = pools["smallp"]; dsum = pools["dsum"]
    onesp = pools["onesp"]; stps = pools["stps"]; projps = pools["projps"]
    ctps = pools["ctps"]; sumps = pools["sumps"]

    ones = onesp.tile([P, P], BF16)
    nc.vector.memset(ones[:], 1.0)

    def _load_w(h):
        wq_sb = wqk.tile([P, DT, E], BF16, tag="wq")
        nc.sync.dma_start(wq_sb[:], wqT[h].rearrange("(c p) e -> p c e", p=P))
        wk_sb = wqk.tile([P, DT, E], BF16, tag="wk")
        nc.sync.dma_start(wk_sb[:], wkT[h].rearrange("(c p) e -> p c e", p=P))
        return wq_sb, wk_sb

    def _load_wv(quad):
        wv_sb = wvp.tile([P, DT, 4 * E], BF16, tag="wv")
        wvr = wvT[quad].rearrange("(c p) e -> p c e", p=P)
        for c4 in range(0, DT, 4):
            nc.sync.dma_start(wv_sb[:, c4:c4 + 4, :], wvr[:, c4:c4 + 4, :])
        return wv_sb

    # DMA issue order tuned for startup: wq0 -> x[0] -> wk0 -> x[1..]
    xTr = xT.rearrange("(c p) t -> p c t", p=P)
    xs = []
    for c in range(DT):
        xt = xpool.tile([P, T], BF16, tag=f"x{c}")
        xs.append(xt)

    wq0_sb = wqk.tile([P, DT, E], BF16, tag="wq")
    wq0r = wqT[0].rearrange("(c p) e -> p c e", p=P)
    wk0_sb = wqk.tile([P, DT, E], BF16, tag="wk")
    wk0r = wkT[0].rearrange("(c p) e -> p c e", p=P)
    nc.sync.dma_start(wq0_sb[:, :4, :], wq0r[:, :4, :])
    nc.sync.dma_start(wk0_sb[:, :4, :], wk0r[:, :4, :])
    nc.sync.dma_start(xs[0][:], xTr[:, 0, :])
    nc.sync.dma_start(wq0_sb[:, 4:, :], wq0r[:, 4:, :])
    nc.sync.dma_start(wk0_sb[:, 4:, :], wk0r[:, 4:, :])
    for c in range(1, DT):
        nc.sync.dma_start(xs[c][:], xTr[:, c, :])
    w_tiles = {0: (wq0_sb, wk0_sb)}
    wv0_sb = _load_wv(0)

    qk_tiles = {}
    v_tiles = {}

    # ---- startup: head-0 Q/K projections (chain-serial, boost banks) ----
    qT0 = qk.tile([P, T], BF16, tag="qT")
    kT0 = qk.tile([P, T], BF16, tag="kT")
    qk_tiles[0] = (qT0, kT0)
    extra = [(ctps, "ct"), (ctps, "ct"), (sumps, "sum"),
             (stps, "st"), (stps, "st"), (stps, "st")]
    ci = 0
    for w_sb, oT in ((wq0_sb, qT0), (wk0_sb, kT0)):
        for nt in range(QC):
            if 1 <= ci <= len(extra):
                pool, tag = extra[ci - 1]
            else:
                pool, tag = projps, "proj"
            ci += 1
            ps = pool.tile([P, QW], F32, tag=tag, name="boot")
            for dt_i in range(DT):
                nc.tensor.matmul(
                    ps[:], lhsT=w_sb[:, dt_i, :],
                    rhs=xs[dt_i][:, nt * QW:(nt + 1) * QW],
                    start=(dt_i == 0), stop=(dt_i == DT - 1))
            nc.vector.tensor_copy(oT[:, nt * QW:(nt + 1) * QW], ps[:])
    w_tiles[1] = _load_w(1)

    # ---- V projection block for a quad (chain-serial) ----
    def _proj_v(quad, wv_sb):
        v_sb = vpool.tile([P, KT, 4 * E], BF16, tag="v", name=f"v{quad}")
        v_tiles[quad] = v_sb
        for kt in range(KT):
            ps = projps.tile([P, 4 * E], F32, tag="proj", name="vps")
            for dt_i in range(DT):
                nc.tensor.matmul(
                    ps[:], lhsT=xs[dt_i][:, kt * P:(kt + 1) * P],
                    rhs=wv_sb[:, dt_i, :],
                    start=(dt_i == 0), stop=(dt_i == DT - 1))
            nc.vector.tensor_copy(v_sb[:, kt, :], ps[:])

    _proj_v(0, wv0_sb)
    wv1_sb = _load_wv(1)

    # ---- background stream: Q/K projection ops for heads 1..7 ----
    def projqk_ops(h):
        wq_sb, wk_sb = w_tiles[h]
        qT = qk.tile([P, T], BF16, tag="qT", name=f"qT{h}")
        kT_sb = qk.tile([P, T], BF16, tag="kT", name=f"kT{h}")
        qk_tiles[h] = (qT, kT_sb)
        order = [(wk_sb, kT_sb, 0), (wq_sb, qT, 0),
                 (wk_sb, kT_sb, 1), (wk_sb, kT_sb, 2), (wk_sb, kT_sb, 3),
                 (wq_sb, qT, 1), (wq_sb, qT, 2), (wq_sb, qT, 3)]
        for w_sb, dest, nt in order:
            ps = projps.tile([P, QW], F32, tag="proj", name="bgp")
            for dt_i in range(DT):
                last = dt_i == DT - 1

                def op(ps=ps, w_sb=w_sb, dt_i=dt_i, nt=nt, dest=dest,
                       last=last):
                    nc.tensor.matmul(
                        ps[:], lhsT=w_sb[:, dt_i, :],
                        rhs=xs[dt_i][:, nt * QW:(nt + 1) * QW],
                        start=(dt_i == 0), stop=last)
                    if last:
                        nc.vector.tensor_copy(
                            dest[:, nt * QW:(nt + 1) * QW], ps[:])
                yield op

    bg = itertools.chain.from_iterable(
        projqk_ops(h) for h in range(1, H_LOC))

    # ---- attention; each slot absorbs `rate` background proj matmuls ----
    def _attn(h, rate):
        quad, hi = divmod(h, 4)
        qT, kT_sb = qk_tiles[h]
        v_sb = v_tiles[quad]
        last_head = h == H_LOC - 1
        for qc in range(QC):
            pe_den = last_head and qc == QC - 1
            if pe_den:
                sm = sumps.tile([P, QW], F32, tag="sum")
            else:
                sm = smallp.tile([P, QW], F32, tag="sm")
            ct = ctps.tile([P, QW], F32, tag="ct")
            pts = [None] * KT
            d12_prev = [None]
            d8_prev = [None]

            def _ct(kt, ct=ct, pts=pts):
                nc.tensor.matmul(
                    ct[:], lhsT=v_sb[:, kt, hi * E:(hi + 1) * E],
                    rhs=pts[kt],
                    start=(kt == 0), stop=(kt == KT - 1))

            for kt in range(KT):
                st = stps.tile([P, QW], F32, tag="st")
                nc.tensor.matmul(
                    st[:], lhsT=kT_sb[:, kt * P:(kt + 1) * P],
                    rhs=qT[:, qc * QW:(qc + 1) * QW],
                    start=True, stop=True)
                pt = ptp.tile([P, QW], BF16, tag="pt")
                nc.scalar.activation(
                    pt[:], st[:], mybir.ActivationFunctionType.Exp,
                    scale=EXP_SCALE)
                pts[kt] = pt[:]
                if kt >= 1:
                    _ct(kt - 1)
                for _ in range(rate):
                    op = next(bg, None)
                    if op is not None:
                        op()
                if kt % 4 == 3:
                    # DVE tree-sums 8 P^T tiles; denominators cost no PE
                    d1 = dsum.tile([P, QW], BF16, tag="d1")
                    nc.vector.tensor_add(d1[:], pts[kt - 3], pts[kt - 2])
                    d2 = dsum.tile([P, QW], BF16, tag="d2")
                    nc.vector.tensor_add(d2[:], pts[kt - 1], pts[kt])
                    d12 = dsum.tile([P, QW], BF16, tag="d12")
                    nc.vector.tensor_add(d12[:], d1[:], d2[:])
                    if kt % 8 == 3:
                        d12_prev[0] = d12
                    else:
                        d8 = dsum.tile([P, QW], BF16, tag="d8")
                        nc.vector.tensor_add(d8[:], d12_prev[0][:], d12[:])
                        if kt == 7:
                            d8_prev[0] = d8
                        else:
                            d16 = dsum.tile([P, QW], BF16, tag="d16")
                            nc.vector.tensor_add(d16[:], d8_prev[0][:], d8[:])
                            if pe_den:
                                nc.tensor.matmul(
                                    sm[:], lhsT=ones[:], rhs=d16[:],
                                    start=True, stop=True)
                            else:
                                nc.gpsimd.partition_all_reduce(
                                    sm[:], d16[:], P, bass_isa.ReduceOp.add)
            _ct(KT - 1)
            rec = smallp.tile([P, QW], F32, tag="rec")
            nc.vector.reciprocal(rec[:], sm[:])
            ot = outp.tile([P, QW], F32, tag="ot")
            nc.vector.tensor_mul(ot[:], ct[:], rec[:])
            nc.sync.dma_start(out[h, :, qc * QW:(qc + 1) * QW], ot[:])

    for h in range(H_LOC):
        if h + 2 < H_LOC:
            w_tiles[h + 2] = _load_w(h + 2)
        if h == H_LOC - 1:
            # everything must be projected by now; drain stragglers
            for op in bg:
                op()
        _attn(h, rate=2 if h < H_LOC - 1 else 0)
        if h == 3:
            _proj_v(1, wv1_sb)


_NC_CACHE = {}


def _get_nc():
    if "nc" not in _NC_CACHE:
        _NC_CACHE["nc"] = _build()
    return _NC_CACHE["nc"]


def _prep_in_maps(x, Wq, Wk, Wv):
    bf = ml_dtypes.bfloat16
    x16 = np.asarray(x).astype(bf)
    Wq16 = np.asarray(Wq).astype(bf)
    Wk16 = np.asarray(Wk).astype(bf)
    Wv16 = np.asarray(Wv).astype(bf)

    xT_by_b = [np.ascontiguousarray(x16[b].T) for b in range(B)]
    wq_by_g, wk_by_g, wv_by_g = [], [], []
    for g in range(2):
        sl = slice(g * H_LOC * E, (g + 1) * H_LOC * E)
        wq_by_g.append(np.ascontiguousarray(
            Wq16[sl].reshape(H_LOC, E, D).transpose(0, 2, 1)))
        wk_by_g.append(np.ascontiguousarray(
            Wk16[sl].reshape(H_LOC, E, D).transpose(0, 2, 1)))
        wv_by_g.append(np.ascontiguousarray(
            Wv16[sl].reshape(H_LOC // 4, 4, E, D)
            .transpose(0, 3, 1, 2).reshape(H_LOC // 4, D, 4 * E)))

    in_maps = []
    for c in range(N_CORES):
        b, g = divmod(c, 2)
        in_maps.append({
            "xT": xT_by_b[b],
            "wqT": wq_by_g[g],
            "wkT": wk_by_g[g],
            "wvT": wv_by_g[g],
        })
    return in_maps


def run_sharded(x, Wq, Wk, Wv, **spmd_kwargs):
    """Build+run on 8 cores; returns (full_output, BassKernelResults)."""
    nc = _get_nc()
    in_maps = _prep_in_maps(x, Wq, Wk, Wv)
    res = run_bass_kernel_spmd(nc, in_maps, list(range(N_CORES)), **spmd_kwargs)
    full = np.empty((B, H, T, E), np.float32)
    for c in range(N_CORES):
        b, g = divmod(c, 2)
        oc = res.results[c]["out"]  # [H_LOC, E, T]
        full[b, g * H_LOC:(g + 1) * H_LOC] = oc.transpose(0, 2, 1)
    return full, res


def kernel(x, Wq, Wk, Wv):
    full, _ = run_sharded(x, Wq, Wk, Wv)
    return full
